# revision 45
# baseline (speedup 1.0000x reference)
"""Trainium2 Bass kernel for the style-modulated encoder layer.

Per batch sample b (data-parallel over B=8 across 8 cores):
  styles = w @ (affine_weight/sqrt(512)).T + affine_bias        [1024]
  s1, s2 = styles[:512], styles[512:]
  xm = x * s1;  xn = instance_norm(xm) over hidden dim (eps=1e-5)
  qd/kd/vd = rsqrt(sum_h (W*s1)^2 + 1e-8); wd likewise with s2
  q = (xn @ qW.T)*qd; k = (xn @ kW.T)*kd; v = (xn @ vW.T)*vd*s2
  o = softmax(q k^T / sqrt(32)) v   (16 heads, depth 32)
  o = (o @ wW.T)*wd + noise_const*noise_strength + bias
  o = leaky_relu(o, 0.2); clip(o, +-256)

Implementation notes (final):
  The O(H^2) scalar prep (styles, demodulation coefficients, noise)
  is folded into the host-side input marshalling: the kernel receives
  s1 / vd*s2 / wd as broadcastable rows and qd/kd as columns, plus the
  four projection weights pre-transposed ([in, out]) in bf16.  On
  device only the heavy work remains: instance-norm over x, the four
  projections, and attention.  All matmul operands are bf16.  ACT runs
  Sqrt (instance norm) and Exp (softmax) only -- two activation-table
  loads.  Attention is blocked qb-outer / head-pair-inner with
  transposed scores [kpos, q]; the o-matmul's lhsT is [V | ones] so
  one PE stream yields both o^T and the softmax row-sum
  (normalised via DVE reciprocal+multiply); the o accumulation is
  bracketed by zero-matmuls so the per-bank PSUM group tolerates the
  interleaved per-head chains.  Attention blocks cover head PAIRS so
  each concurrent tile_position row-tile owns a full PSUM bank
  (same-bank concurrent row-tiles crash the hardware).  Later head groups' q/k projections
  and the per-qb output projections sit in a FIFO drained two items
  per attention kt, keeping the in-order PE queue just ahead of the
  ACT exp stream (the critical resource).  GPSIMD/Pool never touches
  PSUM (hardware restriction); it handles the SBUF-side element-wise
  work instead.
"""

import numpy as np

S = 1024
H = 512
P = 128
HT = H // P          # 4 h-tiles
ST = S // P          # 8 s-tiles
NHEADS = 16
DEPTH = 32
NG = 4               # head groups of 4 heads
QB = 512             # q-block (free dim of transposed scores)
NQB = S // QB
SCALE = DEPTH ** -0.5
CLAMP = 256.0
N_CORES = 8


def _build(nc, mybir, bass, tile, stage=99):
    f32 = mybir.dt.float32
    bf16 = mybir.dt.bfloat16
    Alu = mybir.AluOpType
    Act = mybir.ActivationFunctionType
    from concourse.masks import make_identity
    from concourse.bass import _add_dep_helper

    # ---- DRAM I/O (weights arrive transposed [in, out] in bf16;
    #      style/demod rows+cols are precomputed host-side) ----
    x_d = nc.dram_tensor("x", [S, H], f32, kind="ExternalInput")
    qw_d = nc.dram_tensor("q_weight", [H, H], bf16, kind="ExternalInput")
    kw_d = nc.dram_tensor("k_weight", [H, H], bf16, kind="ExternalInput")
    vw_d = nc.dram_tensor("v_weight", [H, H], bf16, kind="ExternalInput")
    ww_d = nc.dram_tensor("w_weight", [H, H], bf16, kind="ExternalInput")
    s1_d = nc.dram_tensor("s1_row", [1, H], f32, kind="ExternalInput")
    vds2_d = nc.dram_tensor("vds2_row", [1, H], f32, kind="ExternalInput")
    wdr_d = nc.dram_tensor("wdr_row", [1, H], f32, kind="ExternalInput")
    qd_d = nc.dram_tensor("qd_col", [H], f32, kind="ExternalInput")
    kd_d = nc.dram_tensor("kd_col", [H], f32, kind="ExternalInput")
    nsc_d = nc.dram_tensor("noisec", [S], f32, kind="ExternalInput")
    bias_d = nc.dram_tensor("bias", [1, H], f32, kind="ExternalInput")
    out_d = nc.dram_tensor("out", [S, H], bf16, kind="ExternalOutput")

    def bcast_row(dram_ap, n, offset_elems=0):
        # [n] contiguous DRAM -> [128, n] partition-broadcast read AP
        return bass.AP(
            tensor=dram_ap.tensor,
            offset=dram_ap.offset + offset_elems,
            ap=[[0, P], [1, n]],
        )

    def col_ap(dram_ap, ncols, offset_elems=0):
        # flat DRAM -> [128, ncols]; (p, c) = v[c*128 + p]
        return bass.AP(
            tensor=dram_ap.tensor,
            offset=dram_ap.offset + offset_elems,
            ap=[[1, P], [P, ncols]],
        )

    with tile.TileContext(nc) as tc:
        with (
            tc.tile_pool(name="persist", bufs=1) as pp,
            tc.tile_pool(name="work", bufs=3) as wp,
            tc.tile_pool(name="psA", bufs=2, space="PSUM") as psA,
            tc.tile_pool(name="psSC", bufs=2, space="PSUM") as psSC,
            tc.tile_pool(name="psO", bufs=2, space="PSUM") as psO,
        ):
            # ---------------- constants / small loads ----------------
            ident = pp.tile([P, P], f32, tag="ident")
            make_identity(nc, ident)

            eps_n = pp.tile([P, 1], f32, tag="eps_n")
            nc.vector.memset(eps_n, 1e-5)
            zrow = pp.tile([1, P], bf16, tag="zrow")
            nc.vector.memset(zrow, 0.0)
            zrhs = pp.tile([1, QB], bf16, tag="zrhs")
            nc.vector.memset(zrhs, 0.0)

            s1_bc = pp.tile([P, H], f32, tag="s1_bc")
            nc.gpsimd.dma_start(out=s1_bc, in_=bcast_row(s1_d[:], H))
            vds2_bc = pp.tile([P, H], f32, tag="vds2_bc")
            nc.gpsimd.dma_start(out=vds2_bc, in_=bcast_row(vds2_d[:], H))
            wdr_bc = pp.tile([P, H], f32, tag="wdr_bc")
            nc.gpsimd.dma_start(out=wdr_bc, in_=bcast_row(wdr_d[:], H))
            qd_col = pp.tile([P, HT], f32, tag="qd_col")
            nc.gpsimd.dma_start(out=qd_col, in_=col_ap(qd_d[:], HT))
            kd_col = pp.tile([P, HT], f32, tag="kd_col")
            nc.gpsimd.dma_start(out=kd_col, in_=col_ap(kd_d[:], HT))
            noise_col = pp.tile([P, ST], f32, tag="noise_col")
            nc.gpsimd.dma_start(out=noise_col, in_=col_ap(nsc_d[:], ST))
            bias_bc = pp.tile([P, H], f32, tag="bias_bc")
            nc.gpsimd.dma_start(out=bias_bc, in_=bcast_row(bias_d[:], H))
            dcol = {"q": qd_col, "k": kd_col}

            # noise+bias combined per s-tile (keeps the epilogue short);
            # filled lazily via the deferred-work queue (Pool, SBUF-only)
            nb_sb = pp.tile([P, ST, H], f32, tag="nb_sb")

            def nb_fill(sts):
                for st in sts:
                    nc.gpsimd.tensor_scalar(
                        nb_sb[:, st, :], bias_bc, noise_col[:, st:st + 1],
                        None, Alu.add,
                    )

            # ------------- weight loads (one strided DMA each) -------------
            wsrc_map = {"q": qw_d, "k": kw_d, "v": vw_d, "w": ww_d}
            wT = {}
            for name in ("v", "q", "k", "w"):
                wT_sb = pp.tile([P, HT, H], bf16, tag=f"wT_{name}",
                                name=f"wT_{name}")
                nc.scalar.dma_start(
                    out=wT_sb,
                    in_=bass.AP(
                        tensor=wsrc_map[name][:].tensor, offset=0,
                        ap=[[H, P], [P * H, HT], [1, H]],
                    ),
                )
                wT[name] = wT_sb

            if stage <= 1:
                nc.gpsimd.dma_start(out=out_d[0:P, :], in_=s1_bc)
                return nc

            # ------------- x: modulate + norm + transpose (pipelined) ------
            # Pool does the SBUF-side element-wise work; DVE owns stats and
            # all PSUM evacuations (Pool cannot access PSUM on hardware).
            xnT = pp.tile([P, HT, S], bf16, tag="xnT")
            x_sb = pp.tile([P, ST, H], f32, tag="x_sb")

            def x_phase(st):
                eng = nc.gpsimd if st % 2 else nc.vector
                nc.sync.dma_start(
                    out=x_sb[:, st, :], in_=x_d[st * P:(st + 1) * P, :]
                )
                eng.tensor_tensor(
                    x_sb[:, st, :], x_sb[:, st, :], s1_bc, Alu.mult
                )
                stats = wp.tile([P, 6], f32, tag="bn_stats")
                nc.vector.bn_stats(out=stats, in_=x_sb[:, st, :])
                mv = wp.tile([P, 2], f32, tag="bn_mv")
                nc.vector.bn_aggr(out=mv, in_=stats)
                # rstd = 1/sqrt(var+eps)
                nc.scalar.activation(
                    out=mv[:, 1:2], in_=mv[:, 1:2], func=Act.Sqrt, bias=eps_n
                )
                nc.vector.reciprocal(out=mv[:, 1:2], in_=mv[:, 1:2])
                # xn = (x - mu) * rstd
                xn_t = wp.tile([P, H], f32, tag="xn_t")
                eng.tensor_scalar(
                    xn_t, x_sb[:, st, :], mv[:, 0:1], mv[:, 1:2],
                    Alu.subtract, Alu.mult,
                )
                tp = psA.tile([P, H], f32, tag="ps_s")
                for hc in range(HT):
                    nc.tensor.transpose(
                        tp[:, hc * P:(hc + 1) * P],
                        xn_t[:, hc * P:(hc + 1) * P], ident,
                    )
                # odd tiles evacuate via ACT (idle during the lead) to keep
                # the DVE chain that gates the first exp short
                if st % 2:
                    nc.scalar.activation(
                        out=xnT[:, 0:HT, st * P:(st + 1) * P],
                        in_=tp.rearrange("p (ht c) -> p ht c", ht=HT),
                        func=Act.Copy,
                    )
                else:
                    nc.vector.tensor_copy(
                        out=xnT[:, 0:HT, st * P:(st + 1) * P],
                        in_=tp.rearrange("p (ht c) -> p ht c", ht=HT),
                    )

            vo_sb = pp.tile([P, ST, NHEADS, 2 * DEPTH], bf16, tag="vo_sb")
            nc.gpsimd.memset(
                vo_sb[:, :, :, DEPTH:2 * DEPTH].rearrange(
                    "p st h c -> p (st h) c"
                ),
                1.0,
            )

            def v_proj(st):
                ps = psA.tile([P, H], f32, tag="ps_s")
                for ht in range(HT):
                    nc.tensor.matmul(
                        ps,
                        xnT[:, ht, st * P:(st + 1) * P],
                        wT["v"][:, ht, :],
                        start=(ht == 0), stop=(ht == HT - 1),
                    )
                nc.vector.tensor_tensor(
                    vo_sb[:, st, :, 0:DEPTH],
                    ps.rearrange("p (h c) -> p h c", h=NHEADS),
                    vds2_bc.rearrange("p (h c) -> p h c", h=NHEADS),
                    Alu.mult,
                )

            q_sb = pp.tile([P, NG, S], bf16, tag="q_sb")
            k_sb = pp.tile([P, NG, S], bf16, tag="k_sb")

            def qk_items(g, name, sb):
                # closures: 4 accumulating matmuls + 1 demod write
                dst = q_sb if name == "q" else k_sb
                state = {}

                def mk(ht):
                    def go():
                        if ht == 0:
                            state["ps"] = psA.tile([P, H], f32, tag="ps_s",
                                                   name="ps_c")
                        nc.tensor.matmul(
                            state["ps"],
                            wT[name][:, ht, g * P:(g + 1) * P],
                            xnT[:, ht, sb * H:(sb + 1) * H],
                            start=(ht == 0), stop=(ht == HT - 1),
                        )
                    return go

                def wr():
                    nc.vector.tensor_scalar(
                        dst[:, g, sb * H:(sb + 1) * H], state["ps"],
                        dcol[name][:, g:g + 1], None, Alu.mult,
                    )

                return [mk(ht) for ht in range(HT)] + [wr]

            def qk_proj(g, name, sb):
                for it in qk_items(g, name, sb):
                    it()

            # ---------------- attention building blocks ----------------
            # head-PAIR blocks: concurrent tile_position row-tiles crash on
            # hardware unless each lands in a distinct PSUM bank, so each
            # block covers two heads whose [128, 512] score slices each own
            # a full bank.
            oT = pp.tile([P, NG, S], bf16, tag="oT")
            bg = []          # deferred work FIFO, two items popped per kt

            def bg_pop(n):
                for _ in range(n):
                    if bg:
                        bg.pop(0)()

            def attn_block(qb, g, half):
                expt = wp.tile([P, ST, 2 * QB], bf16, tag="expt", bufs=2)
                oers = psO.tile([P, QB], f32, tag="oers")
                chain = [nc.tensor.matmul(
                    oers, zrow, zrhs, start=True, stop=False
                )]

                def omm(*args, **kwargs):
                    inst = nc.tensor.matmul(*args, **kwargs)
                    _add_dep_helper(
                        inst.ins, chain[-1].ins, sync=False,
                        reason="psum bank group order",
                    )
                    chain[-1] = inst

                for kt in range(ST):
                    sc_ps = psSC.tile([P, 2 * QB], f32, tag="sc")
                    for i in range(2):
                        j = 2 * half + i
                        nc.tensor.matmul(
                            sc_ps[:, i * QB:(i + 1) * QB],
                            k_sb[32 * j:32 * (j + 1), g,
                                 kt * P:(kt + 1) * P],
                            q_sb[32 * j:32 * (j + 1), g,
                                 qb * QB:(qb + 1) * QB],
                            start=True, stop=True,
                            tile_position=(32 * j, 0),
                        )
                    nc.scalar.activation(
                        out=expt[:, kt, :], in_=sc_ps,
                        func=Act.Exp, scale=SCALE,
                    )
                    # o^T and row-sums: lhsT = [V | ones]
                    for i in range(2):
                        j = 2 * half + i
                        omm(
                            oers[64 * i:64 * i + 64, :],
                            vo_sb[:, kt, g * 4 + j, :],
                            expt[:, kt, i * QB:(i + 1) * QB],
                            start=False, stop=False,
                            tile_position=(0, 64 * i),
                        )
                    bg_pop(2)
                omm(oers[:, 0:1], zrow, zrhs[:, 0:1], start=False, stop=True)
                # softmax normalisation: reciprocal of the row-sums into
                # SBUF (DVE divide is unsupported; only one PSUM input is
                # allowed per DVE op), then multiply
                rr_sb = wp.tile([P, QB], f32, tag="rr_sb", name="rr_sb")
                for i in range(2):
                    lo = 64 * i + 32
                    nc.vector.reciprocal(
                        out=rr_sb[lo:lo + 32, :], in_=oers[lo:lo + 32, :]
                    )
                for i in range(2):
                    j = 2 * half + i
                    nc.vector.tensor_tensor(
                        oT[32 * j:32 * (j + 1), g,
                           qb * QB:(qb + 1) * QB],
                        oers[64 * i:64 * i + 32, :],
                        rr_sb[64 * i + 32:64 * i + 64, :],
                        Alu.mult,
                    )

            def outproj_items(st):
                eng = nc.gpsimd if st % 2 else nc.vector
                state = {}

                def mk(g):
                    def go():
                        if g == 0:
                            state["ps"] = psA.tile([P, H], f32, tag="ps_s",
                                                   name="ps_c")
                        nc.tensor.matmul(
                            state["ps"],
                            oT[:, g, st * P:(st + 1) * P],
                            wT["w"][:, g, :],
                            start=(g == 0), stop=(g == NG - 1),
                        )
                    return go

                def epi():
                    ps = state["ps"]
                    t1 = wp.tile([P, H], f32, tag="ep_t1", name="ep_t1")
                    nc.vector.tensor_tensor(t1, ps, wdr_bc, Alu.mult)
                    eng.tensor_tensor(t1, t1, nb_sb[:, st, :], Alu.add)
                    t2 = wp.tile([P, H], f32, tag="ep_t2", name="ep_t2")
                    nc.vector.scalar_tensor_tensor(
                        out=t2, in0=t1, scalar=0.2, in1=t1,
                        op0=Alu.mult, op1=Alu.max,
                    )
                    t3 = wp.tile([P, H], bf16, tag="ep_t3", name="ep_t3")
                    eng.tensor_scalar(t3, t2, CLAMP, -CLAMP, Alu.min, Alu.max)
                    nc.sync.dma_start(
                        out=out_d[st * P:(st + 1) * P, :], in_=t3
                    )

                return [mk(g) for g in range(NG)] + [epi]

            # ---------------- dependency-ordered emission ----------------
            for st in range(4):
                x_phase(st)
                if stage > 2:
                    v_proj(st)

            if stage <= 2:
                for st in range(4, ST):
                    x_phase(st)
                xv = wp.tile([P, H], f32, tag="xv")
                nc.vector.tensor_copy(out=xv, in_=xnT[:, 0, 0:H])
                nc.gpsimd.dma_start(out=out_d[0:P, :], in_=xv)
                return nc

            qk_proj(0, "q", 0)
            qk_proj(0, "k", 0)
            for st in range(4, ST):
                x_phase(st)
                v_proj(st)
            qk_proj(0, "q", 1)
            qk_proj(0, "k", 1)

            if stage <= 3:
                qv = wp.tile([P, H], f32, tag="xv")
                nc.vector.tensor_copy(out=qv, in_=q_sb[:, 0, 0:H])
                nc.gpsimd.dma_start(out=out_d[0:P, :], in_=qv)
                return nc

            # deferred-work queue keeps the PE just ahead of the exp stream
            for qb in range(NQB):
                for g in range(NG):
                    for half in range(2):
                        if qb == 0 and half == 0:
                            if g < NG - 1:
                                bg.extend(qk_items(g + 1, "q", 0))
                                bg.extend(qk_items(g + 1, "k", 0))
                                bg.extend(qk_items(g + 1, "k", 1))
                            else:
                                bg.append(lambda: nb_fill(range(4)))
                                bg.append(lambda: nb_fill(range(4, ST)))
                                for gg in range(1, NG):
                                    bg.extend(qk_items(gg, "q", 1))
                        attn_block(qb, g, half)
                if stage <= 4 and qb == 0:
                    while bg:
                        bg.pop(0)()
                    ov = wp.tile([P, H], f32, tag="xv")
                    nc.vector.tensor_copy(out=ov, in_=oT[:, 0, 0:H])
                    nc.gpsimd.dma_start(out=out_d[0:P, :], in_=ov)
                    return nc
                for st in range(4 * qb, 4 * qb + 4):
                    bg.extend(outproj_items(st))

            while bg:
                bg.pop(0)()

    return nc


def build_bass(stage=99):
    import concourse.bass as bass
    import concourse.bacc as bacc
    import concourse.mybir as mybir
    import concourse.tile as tile

    nc = bacc.Bacc()
    _build(nc, mybir, bass, tile, stage)
    nc.compile()
    return nc


def make_in_map(inputs, b):
    import ml_dtypes

    bf16 = ml_dtypes.bfloat16

    def tb(a):  # transpose + bf16, contiguous
        return np.ascontiguousarray(np.asarray(a).T).astype(bf16)

    # host-side scalar prep (styles / demod coefficients), float64
    A = np.asarray(inputs["affine_weight"], np.float64)
    wv = np.asarray(inputs["w"][b], np.float64)
    styles = wv @ (A / np.sqrt(H)).T + np.asarray(inputs["affine_bias"], np.float64)
    s1, s2 = styles[:H], styles[H:]

    def dcoef(W, s):
        W = np.asarray(W, np.float64)
        return 1.0 / np.sqrt(((W * s) ** 2).sum(1) + 1e-8)

    qd = dcoef(inputs["q_weight"], s1)
    kd = dcoef(inputs["k_weight"], s1)
    vd = dcoef(inputs["v_weight"], s1)
    wd = dcoef(inputs["w_weight"], s2)
    noise = (
        np.asarray(inputs["noise_const"], np.float64).reshape(-1)
        * float(np.asarray(inputs["noise_strength"]))
    )

    return {
        "x": np.ascontiguousarray(inputs["x"][b], np.float32),
        "q_weight": tb(inputs["q_weight"]),
        "k_weight": tb(inputs["k_weight"]),
        "v_weight": tb(inputs["v_weight"]),
        "w_weight": tb(inputs["w_weight"]),
        "s1_row": s1.astype(np.float32).reshape(1, H),
        "vds2_row": (vd * s2).astype(np.float32).reshape(1, H),
        "wdr_row": wd.astype(np.float32).reshape(1, H),
        "qd_col": qd.astype(np.float32),
        "kd_col": kd.astype(np.float32),
        "noisec": noise.astype(np.float32),
        "bias": np.asarray(inputs["bias"], np.float32).reshape(1, H),
    }


def kernel(**inputs):
    from concourse.bass_utils import run_bass_kernel_spmd

    nc = build_bass()
    in_maps = [make_in_map(inputs, b) for b in range(N_CORES)]
    res = run_bass_kernel_spmd(nc, in_maps, core_ids=list(range(N_CORES)))
    out = np.stack([res.results[b]["out"] for b in range(N_CORES)], axis=0)
    return out.astype(np.float32)
